# revision 31
# baseline (speedup 1.0000x reference)
"""Trainium2 Bass kernel for nn_CapsuleLayer (dynamic routing capsule layer).

Sharding: the 1152 input capsules (i) are split across 8 cores (144 each);
the full batch B=128 lives on SBUF partitions. Routing state (c, p) stays
local to each core's i-shard; the per-iteration s partial sums are combined
with 3 small AllReduces ([128,160] f32). u_hat is never materialized — both
big contractions are rewritten through W:
  s[b,j,d]       = sum_{i,k} (exp(c)/sigma)[b,j,i] x[b,i,k] W[j,i,d,k]  (PE)
  c_delta[b,j,i] = sum_k x[b,i,k] m[b,j,i,k],   m = sum_d v[b,j,d] W[j,i,d,k]

Cost-model-driven choices:
- One activation table (natural_log_exp_and_others) serves exp/ln/copy;
  sqrt(z) in squash is exp(0.5*ln z) so the Sqrt table is never loaded.
- m-matmuls contract d with a W view [d, (k,i)] and land k-major in PSUM in
  bank-exact N=512 pieces (i<128) plus an N=64 remainder tile, so each j
  needs one Act evacuation; the k-reduction is a fused bf16 tree per j-half
  (first level of half 0 on Pool), with exp(c) issued per half.
- The 16-capsule tail chunk of the s-step runs transpose-free per j: a
  b-space multiply, one PE transpose, one matmul against W1ik (built once
  from the d-major W).
- All PE-transpose evacs share one 1-bank PSUM stage tile; W_D arrives as
  bf16 via gpsimd casting DMAs so v^T / m-matmuls run fully in bf16.
"""

import sys

if "/opt/trn_rl_repo" not in sys.path:
    sys.path.insert(0, "/opt/trn_rl_repo")

import contextlib

import numpy as np

import concourse.bass as bass  # noqa: F401
import concourse.tile as tile
from concourse import bacc, mybir
from concourse.bass_utils import run_bass_kernel_spmd
from concourse.hw_specs import get_activation_tables
from concourse.masks import make_identity

f32 = mybir.dt.float32
f32r = mybir.dt.float32r
bf16 = mybir.dt.bfloat16
AL = mybir.AluOpType
AF = mybir.ActivationFunctionType

B = 128          # batch (on partitions)
NJ = 10          # output capsules
DO = 16          # output capsule dim
DI = 8           # input capsule dim
NI = 1152        # input capsules (global)
ROUTINGS = 3
EPS = 1e-7
NIL0 = 128       # i-split per core: [0:128] main + [128:144] tail


def _pin_act_table(nc):
    # Pre-load the one activation table that serves every func this kernel
    # uses (exp/ln/copy/identity), so the auto-insertion pass doesn't thrash
    # between per-func tables (1283ns per load).
    need = {AF.Exp, AF.Ln, AF.Copy, AF.Identity}
    try:
        tabs = get_activation_tables(nc.m.arch)
    except Exception:
        return
    for idx, funcs in enumerate(tabs.values()):
        if need <= funcs:
            nc.scalar.add_instruction(mybir.InstLoadActFuncSet(
                name=nc.get_next_instruction_name(),
                act_func_set_id=idx,
                engine=mybir.EngineType.Activation,
                ins=[], outs=[]))
            return


def build_kernel(n_cores=8, debug=False, repeat=1, single=False, ablate=()):
    ni_l = NI // n_cores
    chunks = []
    o = 0
    while o < ni_l:
        chunks.append((o, min(128, ni_l - o)))
        o += 128

    nc = bacc.Bacc("TRN2", target_bir_lowering=False, debug=False,
                   num_devices=1 if single else n_cores)
    x_d = nc.dram_tensor("x", [B, ni_l, DI], f32, kind="ExternalInput")
    w_d = nc.dram_tensor("w", [NJ, ni_l, DO, DI], f32, kind="ExternalInput")
    out_d = nc.dram_tensor("out", [B, NJ, DO], f32, kind="ExternalOutput")
    dbg = {}
    if debug:
        dbg["c"] = nc.dram_tensor("dbg_c", [B, NJ, ni_l], f32, kind="ExternalOutput")
        dbg["p"] = nc.dram_tensor("dbg_p", [B, NJ, ni_l], f32, kind="ExternalOutput")
        dbg["s0"] = nc.dram_tensor("dbg_s0", [B, NJ, DO], f32, kind="ExternalOutput")
        dbg["v0"] = nc.dram_tensor("dbg_v0", [B, NJ, DO], f32, kind="ExternalOutput")
        dbg["m0"] = nc.dram_tensor("dbg_m0", [B, DI, NI // 8], f32, kind="ExternalOutput")

    with tile.TileContext(nc) as tc:
        _pin_act_table(nc)
        for _rep in range(repeat):
            _body(nc, tc, x_d, w_d, out_d, dbg if _rep == repeat - 1 else {},
                  ni_l, chunks, n_cores, single, ablate)
    nc.compile()
    return nc


def _body(nc, tc, x_d, w_d, out_d, dbg, ni_l, chunks, n_cores, single=False, ablate=()):
    nt = ni_l - NIL0     # tail width (16)
    ctx = contextlib.ExitStack()
    with ctx:
        sb = ctx.enter_context(tc.tile_pool(name="sb", bufs=1))
        sc = ctx.enter_context(tc.tile_pool(name="scratch", bufs=2))
        ps = ctx.enter_context(tc.tile_pool(name="ps", bufs=2, space="PSUM"))
        ps1 = ctx.enter_context(tc.tile_pool(name="ps1", bufs=1, space="PSUM"))
        dram = ctx.enter_context(tc.tile_pool(name="dram", bufs=1, space="DRAM"))

        def stage():
            # one shared 1-bank PSUM staging tile for PE-transpose evacs;
            # slots 0-7, half-batches reuse slots 0-4
            return ps1.tile([128, 8, B], bf16, tag="stage", name="stage")

        # ---------------- Phase 0: loads + casts ----------------
        ident = sb.tile([128, 128], bf16)
        make_identity(nc, ident)

        x_f = sc.tile([B, ni_l * DI], f32, tag="xload")
        nc.sync.dma_start(out=x_f[:, 0:NIL0 * DI],
                          in_=x_d.ap()[:, 0:NIL0, :].rearrange("b i k -> b (i k)"))
        nc.sync.dma_start(out=x_f[:, NIL0 * DI:],
                          in_=x_d.ap()[:, NIL0:, :].rearrange("b i k -> b (i k)"))

        # natural W: [(i)ch, j, d, k] f32 -> bf16
        w_fs, w_bf = [], []
        for c0, cn in chunks:
            wf = sc.tile([cn, NJ, DO, DI], f32, tag=f"wload{c0}", name="wf")
            for _jh in range(2):
                _js = slice(_jh * NJ // 2, (_jh + 1) * NJ // 2)
                nc.sync.dma_start(
                    out=wf[:, _js, :, :],
                    in_=w_d.ap()[_js, c0:c0 + cn, :, :].rearrange("j i d k -> i j d k"))
            w_fs.append(wf)

        # W_D: [(d)16p, j, i, k] bf16 via gpsimd casting DMA (SWDGE), which
        # also keeps the HWDGE issue slot free for the x/w loads gating s0.
        # Only the first 4 j's load now (their transfers drain before the AR1
        # staging DMAs need the DMA engines); the rest are issued post-AR1.
        w_dT = sb.tile([DO, NJ, ni_l, DI], bf16)
        wdt_barrier = sb.tile([1, 1], f32)
        for _j in range(4):
            nc.gpsimd.dma_start(
                out=w_dT[:, _j, :, :],
                in_=w_d.ap()[_j].rearrange("i d k -> d i k"))

        x_bf = sb.tile([B, ni_l, DI], bf16)
        nc.vector.tensor_copy(
            out=x_bf[:, 0:NIL0, :].rearrange("b i k -> b (i k)"),
            in_=x_f[:, 0:NIL0 * DI])
        nc.vector.tensor_copy(
            out=x_bf[:, NIL0:, :].rearrange("b i k -> b (i k)"),
            in_=x_f[:, NIL0 * DI:])

        for ci, (c0, cn) in enumerate(chunks):
            wb = sb.tile([cn, NJ, DO, DI], bf16, tag=f"wbf{c0}", name="wb")
            for _jh in range(2):
                _js = slice(_jh * NJ // 2, (_jh + 1) * NJ // 2)
                nc.scalar.copy(out=wb[:, _js, :, :], in_=w_fs[ci][:, _js, :, :])
            w_bf.append(wb)

        # x_P: [(i)ch, k, b] bf16 via PE transposes of k-slices; evacs fused
        x_P = [sb.tile([cn, DI, B], bf16, tag=f"xP{c0}", name=f"xP{c0}") for c0, cn in chunks]
        for ci, (c0, cn) in enumerate(chunks):
            pxp = stage()
            for k in range(DI):
                nc.tensor.transpose(pxp[:cn, k, :], x_bf[:, c0:c0 + cn, k], ident)
            nc.vector.tensor_copy(
                out=x_P[ci].rearrange("i k b -> i (k b)"),
                in_=pxp[:cn, :DI, :].rearrange("i k b -> i (k b)"))

        # x in k-major (k,i) order for the c-update multiply (needed only
        # after AR1, so emitted after the s0-critical transposes)
        x_ki = sb.tile([B, DI, ni_l], bf16)
        nc.gpsimd.tensor_copy(out=x_ki, in_=x_bf.rearrange("b i k -> b k i"))

        # ---------------- r0: s0 = (1/NJ) * sum_ik x W ----------------
        ps_s = ps1.tile([B, NJ, DO], f32, tag="smm")
        nmm = len(chunks) * DI * 2
        imm = 0
        for ci, (c0, cn) in enumerate(chunks):
            for jh in range(2):
                js = slice(jh * NJ // 2, (jh + 1) * NJ // 2)
                for k in range(DI):
                    nc.tensor.matmul(
                        ps_s[:, js, :].rearrange("b j d -> b (j d)"),
                        lhsT=x_P[ci][:, k, :],
                        rhs=w_bf[ci][:, js, :, k].rearrange("i j d -> i (j d)"),
                        start=(imm < 2 * DI and k == 0),
                        stop=(imm >= nmm - 2 * DI and k == DI - 1),
                    )
                    imm += 1

        # W1ik: [(i,k)=128p, j, d] bf16 for the transpose-free tail s-step,
        # built from w_dT once it lands (consumed only from softmax #1 on)
        W1ik = sb.tile([nt * DI, NJ, DO], bf16)
        w1st = stage()
        for j in range(NJ):
            sl_s, sl_c = (0, j * DO) if j < 8 else (1, (j - 8) * DO)
            nc.tensor.transpose(
                w1st[:, sl_s, sl_c:sl_c + DO],
                w_dT[:, j, NIL0:, :].rearrange("d i k -> d (i k)"),
                ident[0:DO, 0:DO])
        nc.vector.tensor_copy(
            out=W1ik[:, 0:8, :].rearrange("p j d -> p (j d)"),
            in_=w1st[:, 0, :])
        nc.vector.tensor_copy(
            out=W1ik[:, 8:10, :].rearrange("p j d -> p (j d)"),
            in_=w1st[:, 1, 0:2 * DO])

        # persistent state tiles
        c_t = sb.tile([B, NJ, ni_l], bf16)       # routing logits (j, i)
        s_full = sb.tile([B, NJ, DO], f32)       # all-reduced s
        v_f = sb.tile([B, NJ, DO], f32)          # squashed v
        v_bf = sb.tile([B, NJ, DO], bf16)
        v_T = sb.tile([DO, NJ, B], bf16)         # v transposed [(d), j, b]
        e_bf = sb.tile([B, NJ, ni_l], bf16)      # exp(c)
        ssum = sb.tile([B, 2, ni_l], f32)        # j-tree staging (f32 tail)
        ssum_bf = sb.tile([B, 5, ni_l], bf16)    # j-tree staging (bf16 head)
        rin = sb.tile([B, ni_l], f32)            # 1/ssum
        rin_bf = sb.tile([B, ni_l], bf16)
        eT0 = sb.tile([128, NJ, B], bf16)        # e^T chunk0 [i, j, b]
        # (p is never materialized for the main chunk; dbg reconstructs it)
        rin_T = sb.tile([NIL0, B], bf16)         # (1/sigma)^T
        xs_P = sb.tile([NIL0, DI, B], bf16)      # x_P * rin^T
        p_tail = sb.tile([B, NJ, 16], bf16)      # p for the 16-capsule tail
        t_all = sb.tile([B, NJ, DI, ni_l], bf16)  # m * x scratch, k-major
        m_bf = [sb.tile([B, DI, ni_l], bf16, tag=f"mbf{j}", name=f"mbf{j}")
                for j in range(NJ)]
        y1b = sb.tile([B, nt, DI], bf16)         # tail p*x in b-space
        sq = sb.tile([B, NJ], f32)
        fac = sb.tile([B, NJ], f32)
        den = sb.tile([B, NJ], f32)
        lnt = sb.tile([B, NJ], f32)
        srt = sb.tile([B, NJ], f32)
        eps_t = sb.tile([B, 1], f32)
        nc.vector.memset(eps_t, EPS)

        ar_in = dram.tile([B, DO * NJ], f32)
        ar_out = dram.tile([B, DO * NJ], f32)
        s_part = sb.tile([B, NJ, DO], f32)

        def allreduce_s(src_psum):
            nc.vector.tensor_copy(out=s_part, in_=src_psum)
            nc.sync.dma_start(out=ar_in, in_=s_part.rearrange("b j d -> b (j d)"))
            if single:
                nc.sync.dma_start(out=ar_out, in_=ar_in)
            else:
                nc.gpsimd.collective_compute(
                    "AllReduce", AL.add,
                    ins=[ar_in.opt()], outs=[ar_out.opt()],
                    replica_groups=[list(range(n_cores))],
                )
            nc.sync.dma_start(out=s_full.rearrange("b j d -> b (j d)"), in_=ar_out)

        def squash(last, scale=1.0):
            # v = s*scale * g(||s*scale||), g(z)=z2/(1+z2)/sqrt(z2+eps)
            # sqrt via exp(.5*ln): stays on the pinned ln/exp act table.
            t = sc.tile([B, NJ, DO], f32, tag="sqt")
            nc.vector.tensor_mul(out=t, in0=s_full, in1=s_full)
            nc.vector.tensor_reduce(
                out=sq, in_=t, axis=mybir.AxisListType.X, op=AL.add)
            # sq holds ||s_raw||^2 ; true sq2 = sq*scale^2
            nc.scalar.activation(out=lnt, in_=sq, func=AF.Ln,
                                 bias=eps_t, scale=scale * scale)
            nc.scalar.activation(out=srt, in_=lnt, func=AF.Exp, scale=0.5)
            # den = (1 + sq2) * sqrt(sq2+eps) = sq2*srt + srt
            nc.vector.scalar_tensor_tensor(
                out=den, in0=sq, scalar=scale * scale,
                in1=srt, op0=AL.mult, op1=AL.mult)
            nc.vector.tensor_tensor(out=den, in0=den, in1=srt, op=AL.add)
            nc.vector.reciprocal(out=den, in_=den)
            # fac = sq2 * scale / ((1+sq2)*sqrt) ; extra *scale maps s_raw->v
            nc.vector.scalar_tensor_tensor(
                out=fac, in0=sq, scalar=scale * scale * scale,
                in1=den, op0=AL.mult, op1=AL.mult)
            if last:
                tgt = sb.tile([B, NJ, DO], f32, name="v_out")
                nc.vector.tensor_mul(
                    out=tgt, in0=s_full,
                    in1=fac.unsqueeze(2).broadcast_to([B, NJ, DO]))
                return tgt
            nc.vector.tensor_mul(
                out=v_bf, in0=s_full,
                in1=fac.unsqueeze(2).broadcast_to([B, NJ, DO]))
            # v_T[(d), j, b] via per-j PE transposes into the shared stage
            vst = stage()
            for jh in range(2):
                jsl = slice(jh * NJ // 2, (jh + 1) * NJ // 2)
                for jo, j in enumerate(range(jh * NJ // 2, (jh + 1) * NJ // 2)):
                    nc.tensor.transpose(vst[:DO, jo, :], v_bf[:, j, :], ident)
                nc.vector.tensor_copy(
                    out=v_T[:, jsl, :].rearrange("d j b -> d (j b)"),
                    in_=vst[:DO, 0:NJ // 2, :].rearrange("d j b -> d (j b)"))
            return None

        w_kij = w_dT.rearrange("d j i k -> d j k i")

        def c_update(first):
            # m_j[b,(k,i)] = sum_d v[b,j,d] W[j,:,d,:] ; t = m*x ; k-tree ; +c
            DVE_J = set()
            for jh in range(2):
                jsl = slice(jh * NJ // 2, (jh + 1) * NJ // 2)
                for j in range(jh * NJ // 2, (jh + 1) * NJ // 2):
                    pm = ps.tile([B, 2, 512], f32, tag="pm", name="pm")
                    rem = ps.tile([B, 2, 64], f32, tag="rem", name="rem")
                    for kq in range(2):
                        ksl = slice(4 * kq, 4 * kq + 4)
                        nc.tensor.matmul(
                            pm[:, kq, :],
                            lhsT=v_T[:, j, :],
                            rhs=w_kij[:, j, ksl, 0:NIL0],
                            start=True, stop=True,
                        )
                        nc.tensor.matmul(
                            rem[:, kq, :],
                            lhsT=v_T[:, j, :],
                            rhs=w_kij[:, j, ksl, NIL0:],
                            start=True, stop=True,
                        )
                    pm_v = pm.rearrange("b q (k4 i) -> b q k4 i", k4=4)
                    rem_v = rem.rearrange("b q (k4 i) -> b q k4 i", k4=4)
                    if j in DVE_J:
                        # DVE reads PSUM directly (1x rate but no evac)
                        nc.vector.tensor_tensor(
                            out=t_all[:, j, :, 0:NIL0].rearrange(
                                "b (q k4) i -> b q k4 i", k4=4), in0=pm_v,
                            in1=x_ki[:, :, 0:NIL0].rearrange(
                                "b (q k4) i -> b q k4 i", k4=4), op=AL.mult)
                    else:
                        nc.scalar.copy(
                            out=m_bf[j][:, :, 0:NIL0].rearrange(
                                "b (q k4) i -> b q k4 i", k4=4), in_=pm_v)
                        nc.vector.tensor_tensor(
                            out=t_all[:, j, :, 0:NIL0], in0=m_bf[j][:, :, 0:NIL0],
                            in1=x_ki[:, :, 0:NIL0], op=AL.mult)
                    # tail goes straight from PSUM into t
                    nc.vector.tensor_tensor(
                        out=t_all[:, j, :, NIL0:].rearrange(
                            "b (q k4) i -> b q k4 i", k4=4), in0=rem_v,
                        in1=x_ki[:, :, NIL0:].rearrange(
                            "b (q k4) i -> b q k4 i", k4=4), op=AL.mult)
                if "c_mul" in ablate:
                    continue
                # fused k-tree for this j-half: 8 -> 4 -> 2 -> (+c)
                th = t_all[:, jsl, :, :]
                nc.vector.tensor_tensor(
                    out=th[:, :, 0:4, :], in0=th[:, :, 0:4, :],
                    in1=th[:, :, 4:8, :], op=AL.add)
                nc.vector.tensor_tensor(
                    out=th[:, :, 0:2, :], in0=th[:, :, 0:2, :],
                    in1=th[:, :, 2:4, :], op=AL.add)
                cv = c_t[:, jsl, :]
                if first:
                    nc.vector.tensor_tensor(
                        out=cv, in0=th[:, :, 0, :], in1=th[:, :, 1, :], op=AL.add)
                else:
                    nc.vector.tensor_tensor(
                        out=th[:, :, 0, :], in0=th[:, :, 0, :],
                        in1=th[:, :, 1, :], op=AL.add)
                    nc.vector.tensor_tensor(
                        out=cv, in0=cv, in1=th[:, :, 0, :], op=AL.add)
                # exp for the following softmax, as soon as this half's c lands
                nc.scalar.activation(out=e_bf[:, jsl, :], in_=cv, func=AF.Exp)
                # partial sigma for this half: 5 -> 2 + carry (into ssum_bf)
                j0 = jh * NJ // 2
                nc.vector.tensor_tensor(
                    out=ssum_bf[:, 2 * jh:2 * jh + 2, :],
                    in0=e_bf[:, j0:j0 + 2, :], in1=e_bf[:, j0 + 2:j0 + 4, :],
                    op=AL.add)
                nc.vector.tensor_tensor(
                    out=ssum_bf[:, 2 * jh, :], in0=ssum_bf[:, 2 * jh, :],
                    in1=e_bf[:, j0 + 4, :], op=AL.add)
            if "c_mul" in ablate:
                nc.vector.memset(c_t.rearrange("b j i -> b (j i)"), 0.0)
                for jh in range(2):
                    jsl = slice(jh * NJ // 2, (jh + 1) * NJ // 2)
                    nc.scalar.activation(out=e_bf[:, jsl, :], in_=c_t[:, jsl, :],
                                         func=AF.Exp)

        def softmax_and_s():
            # e was computed at the c_update tail. Transpose e per j (PE) while
            # DVE reduces sigma; fold 1/sigma into x once (xs_P) instead of
            # into every e-transpose.
            pst = stage()
            for jh in range(2):
                jsl = slice(jh * NJ // 2, (jh + 1) * NJ // 2)
                for jo, j in enumerate(range(jh * NJ // 2, (jh + 1) * NJ // 2)):
                    nc.tensor.transpose(pst[:, jo, :], e_bf[:, j, 0:NIL0], ident)
                nc.vector.tensor_copy(
                    out=eT0[:, jsl, :].rearrange("i j b -> i (j b)"),
                    in_=pst[:, 0:NJ // 2, :].rearrange("i j b -> i (j b)"))
            # combine the per-half partial sigmas: (s0+s1) + (s2+s3)
            nc.vector.tensor_tensor(
                out=ssum_bf[:, 0:2, :], in0=ssum_bf[:, 0:2, :],
                in1=ssum_bf[:, 2:4, :], op=AL.add)
            nc.vector.tensor_tensor(
                out=ssum[:, 1, :], in0=ssum_bf[:, 0, :], in1=ssum_bf[:, 1, :],
                op=AL.add)
            nc.vector.reciprocal(out=rin, in_=ssum[:, 1, :])
            nc.vector.tensor_copy(out=rin_bf, in_=rin)
            # rin^T for the main chunk; tail keeps a tiny explicit p
            rst = stage()
            nc.tensor.transpose(rst[:, 5, :], rin_bf[:, 0:NIL0], ident)
            nc.vector.tensor_copy(out=rin_T, in_=rst[:, 5, :])
            nc.vector.tensor_tensor(
                out=xs_P, in0=x_P[0],
                in1=rin_T.unsqueeze(1).broadcast_to([NIL0, DI, B]), op=AL.mult)
            nc.vector.tensor_tensor(
                out=p_tail, in0=e_bf[:, :, NIL0:],
                in1=rin_bf[:, NIL0:].unsqueeze(1).broadcast_to([B, NJ, nt]),
                op=AL.mult)
            if "s_tp" in ablate:
                return None
            ps_sr = ps1.tile([B, NJ, DO], f32, tag="smm", name="ps_sr")
            for j in range(NJ):
                y = sc.tile([NIL0, DI, B], bf16, tag="y0", name="y")
                nc.vector.tensor_tensor(
                    out=y, in0=xs_P,
                    in1=eT0[:, j, :].unsqueeze(1).broadcast_to([NIL0, DI, B]),
                    op=AL.mult)
                # tail: b-space multiply, transpose, single matmul vs W1ik
                nc.gpsimd.tensor_tensor(
                    out=y1b, in0=x_bf[:, NIL0:, :],
                    in1=p_tail[:, j, :].unsqueeze(2).broadcast_to([B, nt, DI]),
                    op=AL.mult)
                nc.tensor.transpose(
                    rst[:, 6 + (j % 2), :],
                    y1b.rearrange("b i k -> b (i k)"), ident)
                y1s = sc.tile([nt * DI, B], bf16, tag="y1s", name="y1s")
                nc.vector.tensor_copy(out=y1s, in_=rst[:, 6 + (j % 2), :])
                if "s_mm" in ablate:
                    continue
                for k in range(DI):
                    nc.tensor.matmul(
                        ps_sr[:, j, :],
                        lhsT=y[:, k, :],
                        rhs=w_bf[0][:, j, :, k],
                        start=(k == 0), stop=False,
                    )
                nc.tensor.matmul(
                    ps_sr[:, j, :], lhsT=y1s, rhs=W1ik[:, j, :],
                    start=False, stop=True,
                )
            return ps_sr

        # ---------------- routing ----------------
        allreduce_s(ps_s)      # r0 s (raw sum; 1/NJ folded into squash)
        # late W_D loads (issued here; transfers mostly clear of AR1 legs)
        for _j in range(4, NJ):
            nc.gpsimd.dma_start(
                out=w_dT[:, _j, :, :],
                in_=w_d.ap()[_j].rearrange("i d k -> d i k"))
        squash(last=False, scale=1.0 / NJ)   # r0 v
        if dbg:
            nc.vector.tensor_copy(out=v_f, in_=v_bf)
            nc.sync.dma_start(out=dbg["v0"].ap(), in_=v_f)
        if "cupd" not in ablate:
            c_update(first=True)   # c1
        if dbg:
            m0_f = sb.tile([B, DI, ni_l], f32)
            nc.vector.tensor_copy(out=m0_f, in_=m_bf[0])
            nc.sync.dma_start(out=dbg["m0"].ap(), in_=m0_f)
        v_out = None
        for r in range(1, ROUTINGS):
            last = (r == ROUTINGS - 1)
            src = softmax_and_s() if "smax" not in ablate else ps_s
            allreduce_s(src)
            v_out = squash(last=last)
            if not last and "cupd" not in ablate:
                c_update(first=False)
        if dbg:
            c_f = sb.tile([B, NJ, ni_l], f32)
            nc.vector.tensor_copy(out=c_f, in_=c_t)
            nc.sync.dma_start(out=dbg["c"].ap(), in_=c_f)
            p_f = sb.tile([B, NJ, ni_l], f32)
            nc.vector.tensor_tensor(
                out=p_f, in0=e_bf,
                in1=rin_bf.unsqueeze(1).broadcast_to([B, NJ, ni_l]), op=AL.mult)
            nc.sync.dma_start(out=dbg["p"].ap(), in_=p_f)
            nc.sync.dma_start(out=dbg["s0"].ap(), in_=s_full)

        nc.sync.dma_start(out=out_d.ap(), in_=v_out)


_NC_CACHE = {}


def kernel(inputs: np.ndarray, W: np.ndarray) -> np.ndarray:
    n_cores = 8
    ni_l = NI // n_cores
    if "nc" not in _NC_CACHE:
        _NC_CACHE["nc"] = build_kernel(n_cores=n_cores, debug=False)
    nc = _NC_CACHE["nc"]
    in_maps = []
    for r in range(n_cores):
        sl = slice(ni_l * r, ni_l * (r + 1))
        in_maps.append({
            "x": np.ascontiguousarray(inputs[:, sl, :], dtype=np.float32),
            "w": np.ascontiguousarray(W[:, sl, :, :], dtype=np.float32),
        })
    res = run_bass_kernel_spmd(nc, in_maps, core_ids=list(range(n_cores)))
    return res.results[0]["out"]


# revision 32
# speedup vs baseline: 1.0050x; 1.0050x over previous
"""Trainium2 Bass kernel for nn_CapsuleLayer (dynamic routing capsule layer).

Sharding: the 1152 input capsules (i) are split across 8 cores (144 each);
the full batch B=128 lives on SBUF partitions. Routing state (c, p) stays
local to each core's i-shard; the per-iteration s partial sums are combined
with 3 small AllReduces ([128,160] f32). u_hat is never materialized — both
big contractions are rewritten through W:
  s[b,j,d]       = sum_{i,k} (exp(c)/sigma)[b,j,i] x[b,i,k] W[j,i,d,k]  (PE)
  c_delta[b,j,i] = sum_k x[b,i,k] m[b,j,i,k],   m = sum_d v[b,j,d] W[j,i,d,k]

Cost-model-driven choices:
- One activation table (natural_log_exp_and_others) serves exp/ln/copy;
  sqrt(z) in squash is exp(0.5*ln z) so the Sqrt table is never loaded.
- m-matmuls contract d with a W view [d, (k,i)] and land k-major in PSUM in
  bank-exact N=512 pieces (i<128) plus an N=64 remainder tile, so each j
  needs one Act evacuation; the k-reduction is a fused bf16 tree per j-half
  (first level of half 0 on Pool), with exp(c) issued per half.
- The 16-capsule tail chunk of the s-step runs transpose-free per j: a
  b-space multiply, one PE transpose, one matmul against W1ik (built once
  from the d-major W).
- All PE-transpose evacs share one 1-bank PSUM stage tile; W_D arrives as
  bf16 via gpsimd casting DMAs so v^T / m-matmuls run fully in bf16.
"""

import sys

if "/opt/trn_rl_repo" not in sys.path:
    sys.path.insert(0, "/opt/trn_rl_repo")

import contextlib

import numpy as np

import concourse.bass as bass  # noqa: F401
import concourse.tile as tile
from concourse import bacc, mybir
from concourse.bass_utils import run_bass_kernel_spmd
from concourse.hw_specs import get_activation_tables
from concourse.masks import make_identity

f32 = mybir.dt.float32
f32r = mybir.dt.float32r
bf16 = mybir.dt.bfloat16
AL = mybir.AluOpType
AF = mybir.ActivationFunctionType

B = 128          # batch (on partitions)
NJ = 10          # output capsules
DO = 16          # output capsule dim
DI = 8           # input capsule dim
NI = 1152        # input capsules (global)
ROUTINGS = 3
EPS = 1e-7
NIL0 = 128       # i-split per core: [0:128] main + [128:144] tail


def _pin_act_table(nc):
    # Pre-load the one activation table that serves every func this kernel
    # uses (exp/ln/copy/identity), so the auto-insertion pass doesn't thrash
    # between per-func tables (1283ns per load).
    need = {AF.Exp, AF.Ln, AF.Copy, AF.Identity}
    try:
        tabs = get_activation_tables(nc.m.arch)
    except Exception:
        return
    for idx, funcs in enumerate(tabs.values()):
        if need <= funcs:
            nc.scalar.add_instruction(mybir.InstLoadActFuncSet(
                name=nc.get_next_instruction_name(),
                act_func_set_id=idx,
                engine=mybir.EngineType.Activation,
                ins=[], outs=[]))
            return


def build_kernel(n_cores=8, debug=False, repeat=1, single=False, ablate=()):
    ni_l = NI // n_cores
    chunks = []
    o = 0
    while o < ni_l:
        chunks.append((o, min(128, ni_l - o)))
        o += 128

    nc = bacc.Bacc("TRN2", target_bir_lowering=False, debug=False,
                   num_devices=1 if single else n_cores)
    x_d = nc.dram_tensor("x", [B, ni_l, DI], f32, kind="ExternalInput")
    w_d = nc.dram_tensor("w", [NJ, ni_l, DO, DI], f32, kind="ExternalInput")
    out_d = nc.dram_tensor("out", [B, NJ, DO], f32, kind="ExternalOutput")
    dbg = {}
    if debug:
        dbg["c"] = nc.dram_tensor("dbg_c", [B, NJ, ni_l], f32, kind="ExternalOutput")
        dbg["p"] = nc.dram_tensor("dbg_p", [B, NJ, ni_l], f32, kind="ExternalOutput")
        dbg["s0"] = nc.dram_tensor("dbg_s0", [B, NJ, DO], f32, kind="ExternalOutput")
        dbg["v0"] = nc.dram_tensor("dbg_v0", [B, NJ, DO], f32, kind="ExternalOutput")
        dbg["m0"] = nc.dram_tensor("dbg_m0", [B, DI, NI // 8], f32, kind="ExternalOutput")

    with tile.TileContext(nc) as tc:
        _pin_act_table(nc)
        for _rep in range(repeat):
            _body(nc, tc, x_d, w_d, out_d, dbg if _rep == repeat - 1 else {},
                  ni_l, chunks, n_cores, single, ablate)
    nc.compile()
    return nc


def _body(nc, tc, x_d, w_d, out_d, dbg, ni_l, chunks, n_cores, single=False, ablate=()):
    nt = ni_l - NIL0     # tail width (16)
    ctx = contextlib.ExitStack()
    with ctx:
        sb = ctx.enter_context(tc.tile_pool(name="sb", bufs=1))
        sc = ctx.enter_context(tc.tile_pool(name="scratch", bufs=2))
        ps = ctx.enter_context(tc.tile_pool(name="ps", bufs=2, space="PSUM"))
        ps1 = ctx.enter_context(tc.tile_pool(name="ps1", bufs=1, space="PSUM"))
        dram = ctx.enter_context(tc.tile_pool(name="dram", bufs=1, space="DRAM"))

        def stage():
            # one shared 1-bank PSUM staging tile for PE-transpose evacs;
            # slots 0-7, half-batches reuse slots 0-4
            return ps1.tile([128, 8, B], bf16, tag="stage", name="stage")

        # ---------------- Phase 0: loads + casts ----------------
        ident = sb.tile([128, 128], bf16)
        make_identity(nc, ident)

        x_f = sc.tile([B, ni_l * DI], f32, tag="xload")
        nc.sync.dma_start(out=x_f[:, 0:NIL0 * DI],
                          in_=x_d.ap()[:, 0:NIL0, :].rearrange("b i k -> b (i k)"))
        nc.sync.dma_start(out=x_f[:, NIL0 * DI:],
                          in_=x_d.ap()[:, NIL0:, :].rearrange("b i k -> b (i k)"))

        # natural W: [(i)ch, j, d, k] f32 -> bf16
        w_fs, w_bf = [], []
        for c0, cn in chunks:
            wf = sc.tile([cn, NJ, DO, DI], f32, tag=f"wload{c0}", name="wf")
            for _jh in range(2):
                _js = slice(_jh * NJ // 2, (_jh + 1) * NJ // 2)
                nc.sync.dma_start(
                    out=wf[:, _js, :, :],
                    in_=w_d.ap()[_js, c0:c0 + cn, :, :].rearrange("j i d k -> i j d k"))
            w_fs.append(wf)

        # W_D: [(d)16p, j, i, k] bf16 via gpsimd casting DMA (SWDGE), which
        # also keeps the HWDGE issue slot free for the x/w loads gating s0.
        # Only the first 4 j's load now (their transfers drain before the AR1
        # staging DMAs need the DMA engines); the rest are issued post-AR1.
        w_dT = sb.tile([DO, NJ, ni_l, DI], bf16)
        wdt_barrier = sb.tile([1, 1], f32)
        for _j in range(4):
            nc.gpsimd.dma_start(
                out=w_dT[:, _j, :, :],
                in_=w_d.ap()[_j].rearrange("i d k -> d i k"))

        x_bf = sb.tile([B, ni_l, DI], bf16)
        nc.vector.tensor_copy(
            out=x_bf[:, 0:NIL0, :].rearrange("b i k -> b (i k)"),
            in_=x_f[:, 0:NIL0 * DI])
        nc.vector.tensor_copy(
            out=x_bf[:, NIL0:, :].rearrange("b i k -> b (i k)"),
            in_=x_f[:, NIL0 * DI:])

        for ci, (c0, cn) in enumerate(chunks):
            wb = sb.tile([cn, NJ, DO, DI], bf16, tag=f"wbf{c0}", name="wb")
            for _jh in range(2):
                _js = slice(_jh * NJ // 2, (_jh + 1) * NJ // 2)
                nc.scalar.copy(out=wb[:, _js, :, :], in_=w_fs[ci][:, _js, :, :])
            w_bf.append(wb)

        # x_P: [(i)ch, k, b] bf16 via PE transposes of k-slices; evacs fused
        x_P = [sb.tile([cn, DI, B], bf16, tag=f"xP{c0}", name=f"xP{c0}") for c0, cn in chunks]
        for ci, (c0, cn) in enumerate(chunks):
            pxp = stage()
            for k in range(DI):
                nc.tensor.transpose(pxp[:cn, k, :], x_bf[:, c0:c0 + cn, k], ident)
            nc.vector.tensor_copy(
                out=x_P[ci].rearrange("i k b -> i (k b)"),
                in_=pxp[:cn, :DI, :].rearrange("i k b -> i (k b)"))

        # x in k-major (k,i) order for the c-update multiply (needed only
        # after AR1, so emitted after the s0-critical transposes)
        x_ki = sb.tile([B, DI, ni_l], bf16)
        nc.gpsimd.tensor_copy(out=x_ki, in_=x_bf.rearrange("b i k -> b k i"))

        # ---------------- r0: s0 = (1/NJ) * sum_ik x W ----------------
        ps_s = ps1.tile([B, NJ, DO], f32, tag="smm")
        nmm = len(chunks) * DI * 2
        imm = 0
        for ci, (c0, cn) in enumerate(chunks):
            for jh in range(2):
                js = slice(jh * NJ // 2, (jh + 1) * NJ // 2)
                for k in range(DI):
                    nc.tensor.matmul(
                        ps_s[:, js, :].rearrange("b j d -> b (j d)"),
                        lhsT=x_P[ci][:, k, :],
                        rhs=w_bf[ci][:, js, :, k].rearrange("i j d -> i (j d)"),
                        start=(imm < 2 * DI and k == 0),
                        stop=(imm >= nmm - 2 * DI and k == DI - 1),
                    )
                    imm += 1

        # W1ik: [(i,k)=128p, j, d] bf16 for the transpose-free tail s-step,
        # built from w_dT once it lands (consumed only from softmax #1 on)
        W1ik = sb.tile([nt * DI, NJ, DO], bf16)
        w1st = stage()
        for j in range(NJ):
            sl_s, sl_c = (0, j * DO) if j < 8 else (1, (j - 8) * DO)
            nc.tensor.transpose(
                w1st[:, sl_s, sl_c:sl_c + DO],
                w_dT[:, j, NIL0:, :].rearrange("d i k -> d (i k)"),
                ident[0:DO, 0:DO])
        nc.vector.tensor_copy(
            out=W1ik[:, 0:8, :].rearrange("p j d -> p (j d)"),
            in_=w1st[:, 0, :])
        nc.vector.tensor_copy(
            out=W1ik[:, 8:10, :].rearrange("p j d -> p (j d)"),
            in_=w1st[:, 1, 0:2 * DO])

        # persistent state tiles
        c_t = sb.tile([B, NJ, ni_l], bf16)       # routing logits (j, i)
        s_full = sb.tile([B, NJ, DO], f32)       # all-reduced s
        v_f = sb.tile([B, NJ, DO], f32)          # squashed v
        v_bf = sb.tile([B, NJ, DO], bf16)
        v_T = sb.tile([DO, NJ, B], bf16)         # v transposed [(d), j, b]
        e_bf = sb.tile([B, NJ, ni_l], bf16)      # exp(c)
        ssum = sb.tile([B, 2, ni_l], f32)        # j-tree staging (f32 tail)
        ssum_bf = sb.tile([B, 5, ni_l], bf16)    # j-tree staging (bf16 head)
        rin = sb.tile([B, ni_l], f32)            # 1/ssum
        rin_bf = sb.tile([B, ni_l], bf16)
        eT0 = sb.tile([128, NJ, B], bf16)        # e^T chunk0 [i, j, b]
        # (p is never materialized for the main chunk; dbg reconstructs it)
        rin_T = sb.tile([NIL0, B], bf16)         # (1/sigma)^T
        xs_P = sb.tile([NIL0, DI, B], bf16)      # x_P * rin^T
        p_tail = sb.tile([B, NJ, 16], bf16)      # p for the 16-capsule tail
        t_all = sb.tile([B, NJ, DI, ni_l], bf16)  # m * x scratch, k-major
        m_bf = [sb.tile([B, DI, ni_l], bf16, tag=f"mbf{j}", name=f"mbf{j}")
                for j in range(NJ)]
        y1b = sb.tile([B, nt, DI], bf16)         # tail p*x in b-space
        sq = sb.tile([B, NJ], f32)
        fac = sb.tile([B, NJ], f32)
        den = sb.tile([B, NJ], f32)
        lnt = sb.tile([B, NJ], f32)
        srt = sb.tile([B, NJ], f32)
        eps_t = sb.tile([B, 1], f32)
        nc.vector.memset(eps_t, EPS)

        ar_in = dram.tile([B, DO * NJ], f32)
        ar_out = dram.tile([B, DO * NJ], f32)
        s_part = sb.tile([B, NJ, DO], f32)

        def allreduce_s(src_psum):
            nc.vector.tensor_copy(out=s_part, in_=src_psum)
            nc.sync.dma_start(out=ar_in, in_=s_part.rearrange("b j d -> b (j d)"))
            if single:
                nc.sync.dma_start(out=ar_out, in_=ar_in)
            else:
                nc.gpsimd.collective_compute(
                    "AllReduce", AL.add,
                    ins=[ar_in.opt()], outs=[ar_out.opt()],
                    replica_groups=[list(range(n_cores))],
                )
            nc.sync.dma_start(out=s_full.rearrange("b j d -> b (j d)"), in_=ar_out)

        def squash(last, scale=1.0):
            # v = s*scale * g(||s*scale||), g(z)=z2/(1+z2)/sqrt(z2+eps)
            # sqrt via exp(.5*ln): stays on the pinned ln/exp act table.
            t = sc.tile([B, NJ, DO], f32, tag="sqt")
            nc.vector.tensor_mul(out=t, in0=s_full, in1=s_full)
            nc.vector.tensor_reduce(
                out=sq, in_=t, axis=mybir.AxisListType.X, op=AL.add)
            # sq holds ||s_raw||^2 ; true sq2 = sq*scale^2
            nc.scalar.activation(out=lnt, in_=sq, func=AF.Ln,
                                 bias=eps_t, scale=scale * scale)
            nc.scalar.activation(out=srt, in_=lnt, func=AF.Exp, scale=0.5)
            # den = (1 + sq2) * sqrt(sq2+eps) = sq2*srt + srt
            nc.vector.scalar_tensor_tensor(
                out=den, in0=sq, scalar=scale * scale,
                in1=srt, op0=AL.mult, op1=AL.mult)
            nc.vector.tensor_tensor(out=den, in0=den, in1=srt, op=AL.add)
            nc.vector.reciprocal(out=den, in_=den)
            # fac = sq2 * scale / ((1+sq2)*sqrt) ; extra *scale maps s_raw->v
            nc.vector.scalar_tensor_tensor(
                out=fac, in0=sq, scalar=scale * scale * scale,
                in1=den, op0=AL.mult, op1=AL.mult)
            if last:
                tgt = sb.tile([B, NJ, DO], f32, name="v_out")
                nc.vector.tensor_mul(
                    out=tgt, in0=s_full,
                    in1=fac.unsqueeze(2).broadcast_to([B, NJ, DO]))
                return tgt
            nc.vector.tensor_mul(
                out=v_bf, in0=s_full,
                in1=fac.unsqueeze(2).broadcast_to([B, NJ, DO]))
            # v_T[(d), j, b] via per-j PE transposes into the shared stage
            vst = stage()
            for jh in range(2):
                jsl = slice(jh * NJ // 2, (jh + 1) * NJ // 2)
                for jo, j in enumerate(range(jh * NJ // 2, (jh + 1) * NJ // 2)):
                    nc.tensor.transpose(vst[:DO, jo, :], v_bf[:, j, :], ident)
                nc.vector.tensor_copy(
                    out=v_T[:, jsl, :].rearrange("d j b -> d (j b)"),
                    in_=vst[:DO, 0:NJ // 2, :].rearrange("d j b -> d (j b)"))
            return None

        w_kij = w_dT.rearrange("d j i k -> d j k i")

        def c_update(first):
            # m_j[b,(k,i)] = sum_d v[b,j,d] W[j,:,d,:] ; t = m*x ; k-tree ; +c
            # j-groups finish their tree/exp/sigma-partial/e-transposes
            # incrementally so only the last group's chain is serial.
            groups = [(0, 5), (5, 8), (8, 10)]
            for gi, (g0, g1) in enumerate(groups):
                jsl = slice(g0, g1)
                for j in range(g0, g1):
                    pm = ps.tile([B, 2, 512], f32, tag="pm", name="pm")
                    rem = ps.tile([B, 2, 64], f32, tag="rem", name="rem")
                    for kq in range(2):
                        ksl = slice(4 * kq, 4 * kq + 4)
                        nc.tensor.matmul(
                            pm[:, kq, :],
                            lhsT=v_T[:, j, :],
                            rhs=w_kij[:, j, ksl, 0:NIL0],
                            start=True, stop=True,
                        )
                        nc.tensor.matmul(
                            rem[:, kq, :],
                            lhsT=v_T[:, j, :],
                            rhs=w_kij[:, j, ksl, NIL0:],
                            start=True, stop=True,
                        )
                    pm_v = pm.rearrange("b q (k4 i) -> b q k4 i", k4=4)
                    rem_v = rem.rearrange("b q (k4 i) -> b q k4 i", k4=4)
                    nc.scalar.copy(
                        out=m_bf[j][:, :, 0:NIL0].rearrange(
                            "b (q k4) i -> b q k4 i", k4=4), in_=pm_v)
                    nc.vector.tensor_tensor(
                        out=t_all[:, j, :, 0:NIL0], in0=m_bf[j][:, :, 0:NIL0],
                        in1=x_ki[:, :, 0:NIL0], op=AL.mult)
                    # tail goes straight from PSUM into t
                    nc.vector.tensor_tensor(
                        out=t_all[:, j, :, NIL0:].rearrange(
                            "b (q k4) i -> b q k4 i", k4=4), in0=rem_v,
                        in1=x_ki[:, :, NIL0:].rearrange(
                            "b (q k4) i -> b q k4 i", k4=4), op=AL.mult)
                if "c_mul" in ablate:
                    continue
                # fused k-tree for this group: 8 -> 4 -> 2 -> (+c)
                th = t_all[:, jsl, :, :]
                nc.vector.tensor_tensor(
                    out=th[:, :, 0:4, :], in0=th[:, :, 0:4, :],
                    in1=th[:, :, 4:8, :], op=AL.add)
                nc.vector.tensor_tensor(
                    out=th[:, :, 0:2, :], in0=th[:, :, 0:2, :],
                    in1=th[:, :, 2:4, :], op=AL.add)
                cv = c_t[:, jsl, :]
                if first:
                    nc.vector.tensor_tensor(
                        out=cv, in0=th[:, :, 0, :], in1=th[:, :, 1, :], op=AL.add)
                else:
                    nc.vector.tensor_tensor(
                        out=th[:, :, 0, :], in0=th[:, :, 0, :],
                        in1=th[:, :, 1, :], op=AL.add)
                    nc.vector.tensor_tensor(
                        out=cv, in0=cv, in1=th[:, :, 0, :], op=AL.add)
                # exp for the following softmax, as soon as this group's c lands
                nc.scalar.activation(out=e_bf[:, jsl, :], in_=cv, func=AF.Exp)
                # sigma partial for this group -> ssum_bf slots 0..3
                if gi == 0:
                    nc.vector.tensor_tensor(
                        out=ssum_bf[:, 0:2, :], in0=e_bf[:, 0:2, :],
                        in1=e_bf[:, 2:4, :], op=AL.add)
                    nc.vector.tensor_tensor(
                        out=ssum_bf[:, 0, :], in0=ssum_bf[:, 0, :],
                        in1=e_bf[:, 4, :], op=AL.add)
                elif gi == 1:
                    nc.vector.tensor_tensor(
                        out=ssum_bf[:, 2, :], in0=e_bf[:, 5, :],
                        in1=e_bf[:, 6, :], op=AL.add)
                    nc.vector.tensor_tensor(
                        out=ssum_bf[:, 2, :], in0=ssum_bf[:, 2, :],
                        in1=e_bf[:, 7, :], op=AL.add)
                else:
                    nc.vector.tensor_tensor(
                        out=ssum_bf[:, 3, :], in0=e_bf[:, 8, :],
                        in1=e_bf[:, 9, :], op=AL.add)
                # e^T transposes for the main chunk of this group
                pst = stage()
                for jo, j in enumerate(range(g0, g1)):
                    nc.tensor.transpose(pst[:, jo, :], e_bf[:, j, 0:NIL0], ident)
                nc.vector.tensor_copy(
                    out=eT0[:, jsl, :].rearrange("i j b -> i (j b)"),
                    in_=pst[:, 0:g1 - g0, :].rearrange("i j b -> i (j b)"))
            if "c_mul" in ablate:
                nc.vector.memset(c_t.rearrange("b j i -> b (j i)"), 0.0)
                for jh in range(2):
                    jsl = slice(jh * NJ // 2, (jh + 1) * NJ // 2)
                    nc.scalar.activation(out=e_bf[:, jsl, :], in_=c_t[:, jsl, :],
                                         func=AF.Exp)

        def softmax_and_s():
            # e, its transposes, and sigma partials all landed in c_update.
            # combine the partial sigmas: (s0+s2) + (s1+s3)
            nc.vector.tensor_tensor(
                out=ssum_bf[:, 0:2, :], in0=ssum_bf[:, 0:2, :],
                in1=ssum_bf[:, 2:4, :], op=AL.add)
            nc.vector.tensor_tensor(
                out=ssum[:, 1, :], in0=ssum_bf[:, 0, :], in1=ssum_bf[:, 1, :],
                op=AL.add)
            nc.vector.reciprocal(out=rin, in_=ssum[:, 1, :])
            nc.vector.tensor_copy(out=rin_bf, in_=rin)
            # rin^T for the main chunk; tail keeps a tiny explicit p
            rst = stage()
            nc.tensor.transpose(rst[:, 5, :], rin_bf[:, 0:NIL0], ident)
            nc.vector.tensor_copy(out=rin_T, in_=rst[:, 5, :])
            nc.vector.tensor_tensor(
                out=xs_P, in0=x_P[0],
                in1=rin_T.unsqueeze(1).broadcast_to([NIL0, DI, B]), op=AL.mult)
            nc.vector.tensor_tensor(
                out=p_tail, in0=e_bf[:, :, NIL0:],
                in1=rin_bf[:, NIL0:].unsqueeze(1).broadcast_to([B, NJ, nt]),
                op=AL.mult)
            if "s_tp" in ablate:
                return None
            ps_sr = ps1.tile([B, NJ, DO], f32, tag="smm", name="ps_sr")
            for j in range(NJ):
                y = sc.tile([NIL0, DI, B], bf16, tag="y0", name="y")
                nc.vector.tensor_tensor(
                    out=y, in0=xs_P,
                    in1=eT0[:, j, :].unsqueeze(1).broadcast_to([NIL0, DI, B]),
                    op=AL.mult)
                # tail: b-space multiply, transpose, single matmul vs W1ik
                nc.gpsimd.tensor_tensor(
                    out=y1b, in0=x_bf[:, NIL0:, :],
                    in1=p_tail[:, j, :].unsqueeze(2).broadcast_to([B, nt, DI]),
                    op=AL.mult)
                nc.tensor.transpose(
                    rst[:, 6 + (j % 2), :],
                    y1b.rearrange("b i k -> b (i k)"), ident)
                y1s = sc.tile([nt * DI, B], bf16, tag="y1s", name="y1s")
                nc.vector.tensor_copy(out=y1s, in_=rst[:, 6 + (j % 2), :])
                if "s_mm" in ablate:
                    continue
                for k in range(DI):
                    nc.tensor.matmul(
                        ps_sr[:, j, :],
                        lhsT=y[:, k, :],
                        rhs=w_bf[0][:, j, :, k],
                        start=(k == 0), stop=False,
                    )
                nc.tensor.matmul(
                    ps_sr[:, j, :], lhsT=y1s, rhs=W1ik[:, j, :],
                    start=False, stop=True,
                )
            return ps_sr

        # ---------------- routing ----------------
        allreduce_s(ps_s)      # r0 s (raw sum; 1/NJ folded into squash)
        # late W_D loads (issued here; transfers mostly clear of AR1 legs)
        for _j in range(4, NJ):
            nc.gpsimd.dma_start(
                out=w_dT[:, _j, :, :],
                in_=w_d.ap()[_j].rearrange("i d k -> d i k"))
        squash(last=False, scale=1.0 / NJ)   # r0 v
        if dbg:
            nc.vector.tensor_copy(out=v_f, in_=v_bf)
            nc.sync.dma_start(out=dbg["v0"].ap(), in_=v_f)
        if "cupd" not in ablate:
            c_update(first=True)   # c1
        if dbg:
            m0_f = sb.tile([B, DI, ni_l], f32)
            nc.vector.tensor_copy(out=m0_f, in_=m_bf[0])
            nc.sync.dma_start(out=dbg["m0"].ap(), in_=m0_f)
        v_out = None
        for r in range(1, ROUTINGS):
            last = (r == ROUTINGS - 1)
            src = softmax_and_s() if "smax" not in ablate else ps_s
            allreduce_s(src)
            v_out = squash(last=last)
            if not last and "cupd" not in ablate:
                c_update(first=False)
        if dbg:
            c_f = sb.tile([B, NJ, ni_l], f32)
            nc.vector.tensor_copy(out=c_f, in_=c_t)
            nc.sync.dma_start(out=dbg["c"].ap(), in_=c_f)
            p_f = sb.tile([B, NJ, ni_l], f32)
            nc.vector.tensor_tensor(
                out=p_f, in0=e_bf,
                in1=rin_bf.unsqueeze(1).broadcast_to([B, NJ, ni_l]), op=AL.mult)
            nc.sync.dma_start(out=dbg["p"].ap(), in_=p_f)
            nc.sync.dma_start(out=dbg["s0"].ap(), in_=s_full)

        nc.sync.dma_start(out=out_d.ap(), in_=v_out)


_NC_CACHE = {}


def kernel(inputs: np.ndarray, W: np.ndarray) -> np.ndarray:
    n_cores = 8
    ni_l = NI // n_cores
    if "nc" not in _NC_CACHE:
        _NC_CACHE["nc"] = build_kernel(n_cores=n_cores, debug=False)
    nc = _NC_CACHE["nc"]
    in_maps = []
    for r in range(n_cores):
        sl = slice(ni_l * r, ni_l * (r + 1))
        in_maps.append({
            "x": np.ascontiguousarray(inputs[:, sl, :], dtype=np.float32),
            "w": np.ascontiguousarray(W[:, sl, :, :], dtype=np.float32),
        })
    res = run_bass_kernel_spmd(nc, in_maps, core_ids=list(range(n_cores)))
    return res.results[0]["out"]


# revision 33
# speedup vs baseline: 1.0222x; 1.0171x over previous
"""Trainium2 Bass kernel for nn_CapsuleLayer (dynamic routing capsule layer).

Sharding: the 1152 input capsules (i) are split across 8 cores (144 each);
the full batch B=128 lives on SBUF partitions. Routing state (c, p) stays
local to each core's i-shard; the per-iteration s partial sums are combined
with 3 small AllReduces ([128,160] f32). u_hat is never materialized — both
big contractions are rewritten through W:
  s[b,j,d]       = sum_{i,k} (exp(c)/sigma)[b,j,i] x[b,i,k] W[j,i,d,k]  (PE)
  c_delta[b,j,i] = sum_k x[b,i,k] m[b,j,i,k],   m = sum_d v[b,j,d] W[j,i,d,k]

Cost-model-driven choices:
- One activation table (natural_log_exp_and_others) serves exp/ln/copy;
  sqrt(z) in squash is exp(0.5*ln z) so the Sqrt table is never loaded.
- m-matmuls contract d with a W view [d, (k,i)] and land k-major in PSUM in
  bank-exact N=512 pieces (i<128) plus an N=64 remainder tile, so each j
  needs one Act evacuation; the k-reduction is a fused bf16 tree per j-half
  (first level of half 0 on Pool), with exp(c) issued per half.
- The 16-capsule tail chunk of the s-step runs transpose-free per j: a
  b-space multiply, one PE transpose, one matmul against W1ik (built once
  from the d-major W).
- All PE-transpose evacs share one 1-bank PSUM stage tile; W_D arrives as
  bf16 via gpsimd casting DMAs so v^T / m-matmuls run fully in bf16.
"""

import sys

if "/opt/trn_rl_repo" not in sys.path:
    sys.path.insert(0, "/opt/trn_rl_repo")

import contextlib

import numpy as np

import concourse.bass as bass  # noqa: F401
import concourse.tile as tile
from concourse import bacc, mybir
from concourse.bass_utils import run_bass_kernel_spmd
from concourse.hw_specs import get_activation_tables
from concourse.masks import make_identity

f32 = mybir.dt.float32
f32r = mybir.dt.float32r
bf16 = mybir.dt.bfloat16
AL = mybir.AluOpType
AF = mybir.ActivationFunctionType

B = 128          # batch (on partitions)
NJ = 10          # output capsules
DO = 16          # output capsule dim
DI = 8           # input capsule dim
NI = 1152        # input capsules (global)
ROUTINGS = 3
EPS = 1e-7
NIL0 = 128       # i-split per core: [0:128] main + [128:144] tail


def _pin_act_table(nc):
    # Pre-load the one activation table that serves every func this kernel
    # uses (exp/ln/copy/identity), so the auto-insertion pass doesn't thrash
    # between per-func tables (1283ns per load).
    need = {AF.Exp, AF.Ln, AF.Copy, AF.Identity}
    try:
        tabs = get_activation_tables(nc.m.arch)
    except Exception:
        return
    for idx, funcs in enumerate(tabs.values()):
        if need <= funcs:
            nc.scalar.add_instruction(mybir.InstLoadActFuncSet(
                name=nc.get_next_instruction_name(),
                act_func_set_id=idx,
                engine=mybir.EngineType.Activation,
                ins=[], outs=[]))
            return


def build_kernel(n_cores=8, debug=False, repeat=1, single=False, ablate=()):
    ni_l = NI // n_cores
    chunks = []
    o = 0
    while o < ni_l:
        chunks.append((o, min(128, ni_l - o)))
        o += 128

    nc = bacc.Bacc("TRN2", target_bir_lowering=False, debug=False,
                   num_devices=1 if single else n_cores)
    x_d = nc.dram_tensor("x", [B, ni_l, DI], f32, kind="ExternalInput")
    w_d = nc.dram_tensor("w", [NJ, ni_l, DO, DI], f32, kind="ExternalInput")
    out_d = nc.dram_tensor("out", [B, NJ, DO], f32, kind="ExternalOutput")
    dbg = {}
    if debug:
        dbg["c"] = nc.dram_tensor("dbg_c", [B, NJ, ni_l], f32, kind="ExternalOutput")
        dbg["p"] = nc.dram_tensor("dbg_p", [B, NJ, ni_l], f32, kind="ExternalOutput")
        dbg["s0"] = nc.dram_tensor("dbg_s0", [B, NJ, DO], f32, kind="ExternalOutput")
        dbg["v0"] = nc.dram_tensor("dbg_v0", [B, NJ, DO], f32, kind="ExternalOutput")
        dbg["m0"] = nc.dram_tensor("dbg_m0", [B, DI, NI // 8], f32, kind="ExternalOutput")

    with tile.TileContext(nc) as tc:
        _pin_act_table(nc)
        for _rep in range(repeat):
            _body(nc, tc, x_d, w_d, out_d, dbg if _rep == repeat - 1 else {},
                  ni_l, chunks, n_cores, single, ablate)
    nc.compile()
    return nc


def _body(nc, tc, x_d, w_d, out_d, dbg, ni_l, chunks, n_cores, single=False, ablate=()):
    nt = ni_l - NIL0     # tail width (16)
    ctx = contextlib.ExitStack()
    with ctx:
        sb = ctx.enter_context(tc.tile_pool(name="sb", bufs=1))
        sc = ctx.enter_context(tc.tile_pool(name="scratch", bufs=2))
        ps = ctx.enter_context(tc.tile_pool(name="ps", bufs=2, space="PSUM"))
        ps1 = ctx.enter_context(tc.tile_pool(name="ps1", bufs=1, space="PSUM"))
        dram = ctx.enter_context(tc.tile_pool(name="dram", bufs=1, space="DRAM"))

        def stage():
            # one shared 1-bank PSUM staging tile for PE-transpose evacs;
            # slots 0-7, half-batches reuse slots 0-4
            return ps1.tile([128, 8, B], bf16, tag="stage", name="stage")

        # ---------------- Phase 0: loads + casts ----------------
        ident = sb.tile([128, 128], bf16)
        make_identity(nc, ident)

        x_f = sc.tile([B, ni_l * DI], f32, tag="xload")
        nc.sync.dma_start(out=x_f[:, 0:NIL0 * DI],
                          in_=x_d.ap()[:, 0:NIL0, :].rearrange("b i k -> b (i k)"))
        nc.sync.dma_start(out=x_f[:, NIL0 * DI:],
                          in_=x_d.ap()[:, NIL0:, :].rearrange("b i k -> b (i k)"))

        # natural W: [(i)ch, j, d, k] f32 -> bf16
        w_fs, w_bf = [], []
        for c0, cn in chunks:
            wf = sc.tile([cn, NJ, DO, DI], f32, tag=f"wload{c0}", name="wf")
            for _jh in range(2):
                _js = slice(_jh * NJ // 2, (_jh + 1) * NJ // 2)
                nc.sync.dma_start(
                    out=wf[:, _js, :, :],
                    in_=w_d.ap()[_js, c0:c0 + cn, :, :].rearrange("j i d k -> i j d k"))
            w_fs.append(wf)

        # W_D: [(d)16p, j, i, k] bf16 via gpsimd casting DMA (SWDGE), which
        # also keeps the HWDGE issue slot free for the x/w loads gating s0.
        # Only the first 4 j's load now (their transfers drain before the AR1
        # staging DMAs need the DMA engines); the rest are issued post-AR1.
        w_dT = sb.tile([DO, NJ, ni_l, DI], bf16)
        wdt_barrier = sb.tile([1, 1], f32)
        for _j in range(4):
            nc.gpsimd.dma_start(
                out=w_dT[:, _j, :, :],
                in_=w_d.ap()[_j].rearrange("i d k -> d i k"))

        x_bf = sb.tile([B, ni_l, DI], bf16)
        nc.vector.tensor_copy(
            out=x_bf[:, 0:NIL0, :].rearrange("b i k -> b (i k)"),
            in_=x_f[:, 0:NIL0 * DI])
        nc.vector.tensor_copy(
            out=x_bf[:, NIL0:, :].rearrange("b i k -> b (i k)"),
            in_=x_f[:, NIL0 * DI:])

        for ci, (c0, cn) in enumerate(chunks):
            wb = sb.tile([cn, NJ, DO, DI], bf16, tag=f"wbf{c0}", name="wb")
            for _jh in range(2):
                _js = slice(_jh * NJ // 2, (_jh + 1) * NJ // 2)
                nc.scalar.copy(out=wb[:, _js, :, :], in_=w_fs[ci][:, _js, :, :])
            w_bf.append(wb)

        # x_P: [(i)ch, k, b] bf16 via PE transposes of k-slices; evacs fused
        x_P = [sb.tile([cn, DI, B], bf16, tag=f"xP{c0}", name=f"xP{c0}") for c0, cn in chunks]
        for ci, (c0, cn) in enumerate(chunks):
            pxp = stage()
            for k in range(DI):
                nc.tensor.transpose(pxp[:cn, k, :], x_bf[:, c0:c0 + cn, k], ident)
            nc.vector.tensor_copy(
                out=x_P[ci].rearrange("i k b -> i (k b)"),
                in_=pxp[:cn, :DI, :].rearrange("i k b -> i (k b)"))

        # x in k-major (k,i) order for the c-update multiply (needed only
        # after AR1, so emitted after the s0-critical transposes)
        x_ki = sb.tile([B, DI, ni_l], bf16)
        nc.gpsimd.tensor_copy(out=x_ki, in_=x_bf.rearrange("b i k -> b k i"))

        # ---------------- r0: s0 = (1/NJ) * sum_ik x W ----------------
        ps_s = ps1.tile([B, NJ, DO], f32, tag="smm")
        nmm = len(chunks) * DI * 2
        imm = 0
        for ci, (c0, cn) in enumerate(chunks):
            for jh in range(2):
                js = slice(jh * NJ // 2, (jh + 1) * NJ // 2)
                for k in range(DI):
                    nc.tensor.matmul(
                        ps_s[:, js, :].rearrange("b j d -> b (j d)"),
                        lhsT=x_P[ci][:, k, :],
                        rhs=w_bf[ci][:, js, :, k].rearrange("i j d -> i (j d)"),
                        start=(imm < 2 * DI and k == 0),
                        stop=(imm >= nmm - 2 * DI and k == DI - 1),
                    )
                    imm += 1

        # W1ik: [(i,k)=128p, j, d] bf16 for the transpose-free tail s-step,
        # built from w_dT once it lands (consumed only from softmax #1 on)
        W1ik = sb.tile([nt * DI, NJ, DO], bf16)
        w1st = stage()
        for j in range(NJ):
            sl_s, sl_c = (0, j * DO) if j < 8 else (1, (j - 8) * DO)
            nc.tensor.transpose(
                w1st[:, sl_s, sl_c:sl_c + DO],
                w_dT[:, j, NIL0:, :].rearrange("d i k -> d (i k)"),
                ident[0:DO, 0:DO])
        nc.vector.tensor_copy(
            out=W1ik[:, 0:8, :].rearrange("p j d -> p (j d)"),
            in_=w1st[:, 0, :])
        nc.vector.tensor_copy(
            out=W1ik[:, 8:10, :].rearrange("p j d -> p (j d)"),
            in_=w1st[:, 1, 0:2 * DO])

        # persistent state tiles
        c_t = sb.tile([B, NJ, ni_l], bf16)       # routing logits (j, i)
        s_full = sb.tile([B, NJ, DO], f32)       # all-reduced s
        v_f = sb.tile([B, NJ, DO], f32)          # squashed v
        v_bf = sb.tile([B, NJ, DO], bf16)
        v_T = sb.tile([DO, NJ, B], bf16)         # v transposed [(d), j, b]
        e_bf = sb.tile([B, NJ, ni_l], bf16)      # exp(c)
        ssum = sb.tile([B, 2, ni_l], f32)        # j-tree staging (f32 tail)
        ssum_bf = sb.tile([B, 5, ni_l], bf16)    # j-tree staging (bf16 head)
        rin = sb.tile([B, ni_l], f32)            # 1/ssum
        rin_bf = sb.tile([B, ni_l], bf16)
        eT0 = sb.tile([128, NJ, B], bf16)        # e^T chunk0 [i, j, b]
        # (p is never materialized for the main chunk; dbg reconstructs it)
        rin_T = sb.tile([NIL0, B], bf16)         # (1/sigma)^T
        xs_P = sb.tile([NIL0, DI, B], bf16)      # x_P * rin^T
        p_tail = sb.tile([B, NJ, 16], bf16)      # p for the 16-capsule tail
        t_all = sb.tile([B, NJ, DI, ni_l], bf16)  # m * x scratch, k-major
        m_bf = [sb.tile([B, DI, ni_l], bf16, tag=f"mbf{j}", name=f"mbf{j}")
                for j in range(NJ)]
        y1b = sb.tile([B, nt, DI], bf16)         # tail p*x in b-space
        sq = sb.tile([B, NJ], f32)
        fac = sb.tile([B, NJ], f32)
        den = sb.tile([B, NJ], f32)
        lnt = sb.tile([B, NJ], f32)
        srt = sb.tile([B, NJ], f32)
        eps_t = sb.tile([B, 1], f32)
        nc.vector.memset(eps_t, EPS)

        ar_in = dram.tile([B, DO * NJ], f32)
        ar_out = dram.tile([B, DO * NJ], f32)
        s_part = sb.tile([B, NJ, DO], f32)

        def allreduce_s(src_psum):
            nc.vector.tensor_copy(out=s_part, in_=src_psum)
            nc.sync.dma_start(out=ar_in, in_=s_part.rearrange("b j d -> b (j d)"))
            if single:
                nc.sync.dma_start(out=ar_out, in_=ar_in)
            else:
                nc.gpsimd.collective_compute(
                    "AllReduce", AL.add,
                    ins=[ar_in.opt()], outs=[ar_out.opt()],
                    replica_groups=[list(range(n_cores))],
                )
            nc.sync.dma_start(out=s_full.rearrange("b j d -> b (j d)"), in_=ar_out)

        def squash(last, scale=1.0):
            # v = s*scale * g(||s*scale||), g(z)=z2/(1+z2)/sqrt(z2+eps)
            # sqrt via exp(.5*ln): stays on the pinned ln/exp act table.
            t = sc.tile([B, NJ, DO], f32, tag="sqt")
            nc.vector.tensor_mul(out=t, in0=s_full, in1=s_full)
            nc.vector.tensor_reduce(
                out=sq, in_=t, axis=mybir.AxisListType.X, op=AL.add)
            # sq holds ||s_raw||^2 ; true sq2 = sq*scale^2
            nc.scalar.activation(out=lnt, in_=sq, func=AF.Ln,
                                 bias=eps_t, scale=scale * scale)
            nc.scalar.activation(out=srt, in_=lnt, func=AF.Exp, scale=0.5)
            # den = (1 + sq2) * sqrt(sq2+eps) = sq2*srt + srt
            nc.vector.scalar_tensor_tensor(
                out=den, in0=sq, scalar=scale * scale,
                in1=srt, op0=AL.mult, op1=AL.mult)
            nc.vector.tensor_tensor(out=den, in0=den, in1=srt, op=AL.add)
            nc.vector.reciprocal(out=den, in_=den)
            # fac = sq2 * scale / ((1+sq2)*sqrt) ; extra *scale maps s_raw->v
            nc.vector.scalar_tensor_tensor(
                out=fac, in0=sq, scalar=scale * scale * scale,
                in1=den, op0=AL.mult, op1=AL.mult)
            if last:
                tgt = sb.tile([B, NJ, DO], f32, name="v_out")
                nc.vector.tensor_mul(
                    out=tgt, in0=s_full,
                    in1=fac.unsqueeze(2).broadcast_to([B, NJ, DO]))
                return tgt
            nc.vector.tensor_mul(
                out=v_bf, in0=s_full,
                in1=fac.unsqueeze(2).broadcast_to([B, NJ, DO]))
            # v_T[(d), j, b] via per-j PE transposes into the shared stage
            vst = stage()
            for jh in range(2):
                jsl = slice(jh * NJ // 2, (jh + 1) * NJ // 2)
                for jo, j in enumerate(range(jh * NJ // 2, (jh + 1) * NJ // 2)):
                    nc.tensor.transpose(vst[:DO, jo, :], v_bf[:, j, :], ident)
                nc.vector.tensor_copy(
                    out=v_T[:, jsl, :].rearrange("d j b -> d (j b)"),
                    in_=vst[:DO, 0:NJ // 2, :].rearrange("d j b -> d (j b)"))
            return None

        w_kij = w_dT.rearrange("d j i k -> d j k i")

        def c_update(first):
            # m_j[b,(k,i)] = sum_d v[b,j,d] W[j,:,d,:] ; t = m*x ; k-tree ; +c
            # j-groups finish their tree/exp/sigma-partial/e-transposes
            # incrementally so only the last group's chain is serial.
            groups = [(0, 5), (5, 8), (8, 10)]
            for gi, (g0, g1) in enumerate(groups):
                jsl = slice(g0, g1)
                for j in range(g0, g1):
                    pm = ps.tile([B, 2, 512], f32, tag="pm", name="pm")
                    rem = ps.tile([B, 2, 64], f32, tag="rem", name="rem")
                    for kq in range(2):
                        ksl = slice(4 * kq, 4 * kq + 4)
                        nc.tensor.matmul(
                            pm[:, kq, :],
                            lhsT=v_T[:, j, :],
                            rhs=w_kij[:, j, ksl, 0:NIL0],
                            start=True, stop=True,
                        )
                        nc.tensor.matmul(
                            rem[:, kq, :],
                            lhsT=v_T[:, j, :],
                            rhs=w_kij[:, j, ksl, NIL0:],
                            start=True, stop=True,
                        )
                    pm_v = pm.rearrange("b q (k4 i) -> b q k4 i", k4=4)
                    rem_v = rem.rearrange("b q (k4 i) -> b q k4 i", k4=4)
                    nc.scalar.copy(
                        out=m_bf[j][:, :, 0:NIL0].rearrange(
                            "b (q k4) i -> b q k4 i", k4=4), in_=pm_v)
                    nc.vector.tensor_tensor(
                        out=t_all[:, j, :, 0:NIL0], in0=m_bf[j][:, :, 0:NIL0],
                        in1=x_ki[:, :, 0:NIL0], op=AL.mult)
                    # tail goes straight from PSUM into t
                    nc.vector.tensor_tensor(
                        out=t_all[:, j, :, NIL0:].rearrange(
                            "b (q k4) i -> b q k4 i", k4=4), in0=rem_v,
                        in1=x_ki[:, :, NIL0:].rearrange(
                            "b (q k4) i -> b q k4 i", k4=4), op=AL.mult)
                if "c_mul" in ablate:
                    continue
                # fused k-tree for this group: 8 -> 4 -> 2 -> (+c)
                th = t_all[:, jsl, :, :]
                nc.vector.tensor_tensor(
                    out=th[:, :, 0:4, :], in0=th[:, :, 0:4, :],
                    in1=th[:, :, 4:8, :], op=AL.add)
                nc.vector.tensor_tensor(
                    out=th[:, :, 0:2, :], in0=th[:, :, 0:2, :],
                    in1=th[:, :, 2:4, :], op=AL.add)
                cv = c_t[:, jsl, :]
                if first:
                    nc.vector.tensor_tensor(
                        out=cv, in0=th[:, :, 0, :], in1=th[:, :, 1, :], op=AL.add)
                else:
                    nc.vector.tensor_tensor(
                        out=th[:, :, 0, :], in0=th[:, :, 0, :],
                        in1=th[:, :, 1, :], op=AL.add)
                    nc.vector.tensor_tensor(
                        out=cv, in0=cv, in1=th[:, :, 0, :], op=AL.add)
                # exp for the following softmax, as soon as this group's c lands
                nc.scalar.activation(out=e_bf[:, jsl, :], in_=cv, func=AF.Exp)
                # sigma partial for this group -> ssum_bf slots 0..3
                if gi == 0:
                    nc.gpsimd.tensor_tensor(
                        out=ssum_bf[:, 0:2, :], in0=e_bf[:, 0:2, :],
                        in1=e_bf[:, 2:4, :], op=AL.add)
                    nc.gpsimd.tensor_tensor(
                        out=ssum_bf[:, 0, :], in0=ssum_bf[:, 0, :],
                        in1=e_bf[:, 4, :], op=AL.add)
                elif gi == 1:
                    nc.gpsimd.tensor_tensor(
                        out=ssum_bf[:, 2, :], in0=e_bf[:, 5, :],
                        in1=e_bf[:, 6, :], op=AL.add)
                    nc.gpsimd.tensor_tensor(
                        out=ssum_bf[:, 2, :], in0=ssum_bf[:, 2, :],
                        in1=e_bf[:, 7, :], op=AL.add)
                else:
                    nc.vector.tensor_tensor(
                        out=ssum_bf[:, 3, :], in0=e_bf[:, 8, :],
                        in1=e_bf[:, 9, :], op=AL.add)
                # e^T transposes for the main chunk of this group
                pst = stage()
                for jo, j in enumerate(range(g0, g1)):
                    nc.tensor.transpose(pst[:, jo, :], e_bf[:, j, 0:NIL0], ident)
                nc.scalar.copy(
                    out=eT0[:, jsl, :].rearrange("i j b -> i (j b)"),
                    in_=pst[:, 0:g1 - g0, :].rearrange("i j b -> i (j b)"))
            if "c_mul" in ablate:
                nc.vector.memset(c_t.rearrange("b j i -> b (j i)"), 0.0)
                for jh in range(2):
                    jsl = slice(jh * NJ // 2, (jh + 1) * NJ // 2)
                    nc.scalar.activation(out=e_bf[:, jsl, :], in_=c_t[:, jsl, :],
                                         func=AF.Exp)

        def softmax_and_s():
            # e, its transposes, and sigma partials all landed in c_update.
            # combine the partial sigmas: (s0+s2) + (s1+s3)
            nc.vector.tensor_tensor(
                out=ssum_bf[:, 0:2, :], in0=ssum_bf[:, 0:2, :],
                in1=ssum_bf[:, 2:4, :], op=AL.add)
            nc.vector.tensor_tensor(
                out=ssum[:, 1, :], in0=ssum_bf[:, 0, :], in1=ssum_bf[:, 1, :],
                op=AL.add)
            nc.vector.reciprocal(out=rin, in_=ssum[:, 1, :])
            nc.vector.tensor_copy(out=rin_bf, in_=rin)
            # rin^T for the main chunk; tail keeps a tiny explicit p
            rst = stage()
            nc.tensor.transpose(rst[:, 5, :], rin_bf[:, 0:NIL0], ident)
            nc.vector.tensor_copy(out=rin_T, in_=rst[:, 5, :])
            nc.vector.tensor_tensor(
                out=xs_P, in0=x_P[0],
                in1=rin_T.unsqueeze(1).broadcast_to([NIL0, DI, B]), op=AL.mult)
            nc.gpsimd.tensor_tensor(
                out=p_tail, in0=e_bf[:, :, NIL0:],
                in1=rin_bf[:, NIL0:].unsqueeze(1).broadcast_to([B, NJ, nt]),
                op=AL.mult)
            if "s_tp" in ablate:
                return None
            ps_sr = ps1.tile([B, NJ, DO], f32, tag="smm", name="ps_sr")
            for j in range(NJ):
                y = sc.tile([NIL0, DI, B], bf16, tag="y0", name="y")
                nc.vector.tensor_tensor(
                    out=y, in0=xs_P,
                    in1=eT0[:, j, :].unsqueeze(1).broadcast_to([NIL0, DI, B]),
                    op=AL.mult)
                # tail: b-space multiply, transpose, single matmul vs W1ik
                nc.gpsimd.tensor_tensor(
                    out=y1b, in0=x_bf[:, NIL0:, :],
                    in1=p_tail[:, j, :].unsqueeze(2).broadcast_to([B, nt, DI]),
                    op=AL.mult)
                nc.tensor.transpose(
                    rst[:, 6 + (j % 2), :],
                    y1b.rearrange("b i k -> b (i k)"), ident)
                y1s = sc.tile([nt * DI, B], bf16, tag="y1s", name="y1s")
                nc.vector.tensor_copy(out=y1s, in_=rst[:, 6 + (j % 2), :])
                if "s_mm" in ablate:
                    continue
                for k in range(DI):
                    nc.tensor.matmul(
                        ps_sr[:, j, :],
                        lhsT=y[:, k, :],
                        rhs=w_bf[0][:, j, :, k],
                        start=(k == 0), stop=False,
                    )
                nc.tensor.matmul(
                    ps_sr[:, j, :], lhsT=y1s, rhs=W1ik[:, j, :],
                    start=False, stop=True,
                )
            return ps_sr

        # ---------------- routing ----------------
        allreduce_s(ps_s)      # r0 s (raw sum; 1/NJ folded into squash)
        # late W_D loads (issued here; transfers mostly clear of AR1 legs)
        for _j in range(4, NJ):
            nc.gpsimd.dma_start(
                out=w_dT[:, _j, :, :],
                in_=w_d.ap()[_j].rearrange("i d k -> d i k"))
        squash(last=False, scale=1.0 / NJ)   # r0 v
        if dbg:
            nc.vector.tensor_copy(out=v_f, in_=v_bf)
            nc.sync.dma_start(out=dbg["v0"].ap(), in_=v_f)
        if "cupd" not in ablate:
            c_update(first=True)   # c1
        if dbg:
            m0_f = sb.tile([B, DI, ni_l], f32)
            nc.vector.tensor_copy(out=m0_f, in_=m_bf[0])
            nc.sync.dma_start(out=dbg["m0"].ap(), in_=m0_f)
        v_out = None
        for r in range(1, ROUTINGS):
            last = (r == ROUTINGS - 1)
            src = softmax_and_s() if "smax" not in ablate else ps_s
            allreduce_s(src)
            v_out = squash(last=last)
            if not last and "cupd" not in ablate:
                c_update(first=False)
        if dbg:
            c_f = sb.tile([B, NJ, ni_l], f32)
            nc.vector.tensor_copy(out=c_f, in_=c_t)
            nc.sync.dma_start(out=dbg["c"].ap(), in_=c_f)
            p_f = sb.tile([B, NJ, ni_l], f32)
            nc.vector.tensor_tensor(
                out=p_f, in0=e_bf,
                in1=rin_bf.unsqueeze(1).broadcast_to([B, NJ, ni_l]), op=AL.mult)
            nc.sync.dma_start(out=dbg["p"].ap(), in_=p_f)
            nc.sync.dma_start(out=dbg["s0"].ap(), in_=s_full)

        nc.sync.dma_start(out=out_d.ap(), in_=v_out)


_NC_CACHE = {}


def kernel(inputs: np.ndarray, W: np.ndarray) -> np.ndarray:
    n_cores = 8
    ni_l = NI // n_cores
    if "nc" not in _NC_CACHE:
        _NC_CACHE["nc"] = build_kernel(n_cores=n_cores, debug=False)
    nc = _NC_CACHE["nc"]
    in_maps = []
    for r in range(n_cores):
        sl = slice(ni_l * r, ni_l * (r + 1))
        in_maps.append({
            "x": np.ascontiguousarray(inputs[:, sl, :], dtype=np.float32),
            "w": np.ascontiguousarray(W[:, sl, :, :], dtype=np.float32),
        })
    res = run_bass_kernel_spmd(nc, in_maps, core_ids=list(range(n_cores)))
    return res.results[0]["out"]
